# revision 35
# baseline (speedup 1.0000x reference)
"""BiMambaBlock Trainium2 Bass kernel (low-I/O design).

Sharding: 8 cores = (batch b) x (branch r in {fwd,bwd}) x (d_inner half h).
All call-invariant data (weights, folded biases, A, D, one-hot helpers) is
baked into the NEFF as inline constants in 4 (r,h)-variants; each core picks
its variant on-device with a tiny one-hot selector input, so the only
per-call traffic is a 1MB x-quarter in (AllGathered on-device) and a 1MB
bf16 y-chunk out (ReduceScattered on-device).  The bwd branch's time flip is
done on-device with reversed-AP DVE copies blended by per-core 0/1 scalars
(SPMD-safe: LN stats are flip-equivariant, so xn is normalized first and
then flip-selected).  B/C scan coefficients are broadcast across partitions
with PE one-hot matmuls instead of stride-0 DMA.
"""

import os
import sys

for _p in ("/opt/trn_rl_repo", "/root/.axon_site/_ro/trn_rl_repo"):
    if os.path.isdir(_p) and _p not in sys.path:
        sys.path.insert(0, _p)
        break

import hashlib
import numpy as np
import ml_dtypes

import concourse.bass as bass
import concourse.mybir as mybir
import concourse.tile as tile
from concourse import bacc

BF16 = ml_dtypes.bfloat16
F32 = mybir.dt.float32
BF = mybir.dt.bfloat16

D_MODEL = 1024
D_INNER = 2048
D_STATE = 16
D_CONV = 4
DT_RANK = 64
BATCH, SEQ = 2, 2048
DL = 1024          # local d_inner half per core
NBLK = DL // 128   # 8 d-blocks of 128
NTC = SEQ // 512   # 4 chunks of 512
NMT = SEQ // 128   # 16 time tiles of 128

MULT = mybir.AluOpType.mult
ADD = mybir.AluOpType.add
SUB = mybir.AluOpType.subtract
AF = mybir.ActivationFunctionType


def _rev(ap):
    """Free-dim reversed view of a 2D AP."""
    n = ap.ap[-1][1]
    return bass.AP(tensor=ap.tensor, offset=ap.offset + (n - 1) * ap.ap[-1][0],
                   ap=[list(ap.ap[0]), [-ap.ap[-1][0], n]])


def _variant_consts(inputs):
    """Host: fold weights into the 4 (r,h) inline-const variants."""
    ln_g = np.asarray(inputs["ln_g"], np.float32)
    ln_b = np.asarray(inputs["ln_b"], np.float32)
    proj_w = np.asarray(inputs["proj_w"], np.float32)
    out = []
    for v in range(4):
        r, h = v // 2, v % 2
        p = "fwd" if r == 0 else "bwd"
        in_w = np.asarray(inputs[p + "_in_w"], np.float32)
        conv_w = np.asarray(inputs[p + "_conv_w"], np.float32)
        conv_b = np.asarray(inputs[p + "_conv_b"], np.float32)
        xproj_w = np.asarray(inputs[p + "_xproj_w"], np.float32)
        dt_w = np.asarray(inputs[p + "_dt_w"], np.float32)
        dt_b = np.asarray(inputs[p + "_dt_b"], np.float32)
        A_log = np.asarray(inputs[p + "_A_log"], np.float32)
        Dp = np.asarray(inputs[p + "_D"], np.float32)
        out_w = np.asarray(inputs[p + "_out_w"], np.float32)

        sl = slice(h * DL, (h + 1) * DL)
        W = np.concatenate(
            [in_w[sl], in_w[D_INNER + h * DL:D_INNER + (h + 1) * DL]], 0)
        W = W * ln_g[None, :]
        cb = W @ ln_b
        cb_x, cb_z = cb[:DL], cb[DL:]
        w_inT = np.ascontiguousarray(W.T).astype(BF16)          # [1024, 2048]

        cwl = conv_w[sl]
        conv_b_eff = conv_b[sl] + cb_x * cwl.sum(1)
        conv_w_c = np.ascontiguousarray(
            cwl.reshape(NBLK, 128, D_CONV).transpose(1, 0, 2)
            .reshape(128, NBLK * D_CONV)).astype(np.float32)

        def col(vv):
            return np.ascontiguousarray(vv.reshape(NBLK, 128).T).astype(np.float32)

        A = -np.exp(A_log[sl])
        a_cols = np.ascontiguousarray(
            A.reshape(NBLK, 128, D_STATE).transpose(1, 0, 2)
            .reshape(128, NBLK * D_STATE)).astype(np.float32)

        w_fold = proj_w[:, r * D_MODEL:(r + 1) * D_MODEL] @ out_w[:, sl]
        w_foldT = np.ascontiguousarray(w_fold.T).astype(BF16)   # [1024, 1024]

        xpw = np.ascontiguousarray(xproj_w[:, sl].T).astype(BF16)  # [1024, 96]
        xpw_pack = np.ascontiguousarray(
            xpw.reshape(NBLK, 128, 96).transpose(1, 0, 2)
            .reshape(128, NBLK * 96)).astype(BF16)              # [128, 768]
        dt_wT = np.ascontiguousarray(dt_w[sl].T).astype(BF16)   # [64, 1024]

        cst = np.concatenate([
            conv_w_c,                 # 0:32
            col(conv_b_eff),          # 32:40
            col(cb_z),                # 40:48
            col(dt_b[sl]),            # 48:56
            a_cols,                   # 56:184
            col(Dp[sl]),              # 184:192
        ], axis=1).astype(np.float32)                           # [128, 192]

        out.append({
            "wi": w_inT, "wf": w_foldT, "xp": xpw_pack,
            "dw": dt_wT, "cst": cst,
        })
    return out


def _build_program(inputs, sim=False):
    var = _variant_consts(inputs)

    nc = bacc.Bacc("TRN2", target_bir_lowering=False, debug=False, num_devices=8)

    # ---- per-core external I/O ----
    xT = nc.declare_dram_parameter("xT", [D_MODEL, SEQ], BF, isOutput=False)
    sel = nc.declare_dram_parameter("sel", [128, 8], F32, isOutput=False)
    y_part = nc.declare_dram_parameter("y_part", [SEQ, D_MODEL], BF, isOutput=True)

    # ---- inline consts ----
    wi_c = [nc.inline_tensor(var[v]["wi"], name=f"wi{v}") for v in range(4)]
    wf_c = [nc.inline_tensor(var[v]["wf"], name=f"wf{v}") for v in range(4)]
    xp_c = [nc.inline_tensor(var[v]["xp"], name=f"xp{v}") for v in range(4)]
    dw_c = [nc.inline_tensor(var[v]["dw"], name=f"dw{v}") for v in range(4)]
    cs_c = [nc.inline_tensor(var[v]["cst"], name=f"cs{v}") for v in range(4)]
    ebc_np = np.zeros((96, 32 * 128), np.float32)
    for k in range(32):
        ebc_np[64 + k, k * 128:(k + 1) * 128] = 1.0
    ebc_c = nc.inline_tensor(ebc_np.astype(BF16), name="ebc")

    # ---- internal DRAM ----
    cc_in = nc.dram_tensor("cc_in", [96, SEQ], BF)
    cc_out = nc.dram_tensor("cc_out", [96, SEQ], BF)
    gate_dram = nc.dram_tensor("gate_dram", [DL, SEQ], BF)

    with tile.TileContext(nc) as tc:
        with (
            tc.tile_pool(name="pc", bufs=1) as pc,            # small consts
            tc.tile_pool(name="pvar", bufs=2) as pvar,        # variant staging
            tc.tile_pool(name="pxn", bufs=8) as pxn,          # xn -> av/bv
            tc.tile_pool(name="pchain", bufs=9) as pchain,    # xr -> u -> dt
            tc.tile_pool(name="pmisc", bufs=2) as pmisc,
            tc.tile_pool(name="pgs", bufs=3) as pgs,          # small streams
        ):
            st = pc.tile([128, 8], F32, tag="st", name="st")
            nc.sync.dma_start(out=st, in_=sel[:])
            s_v = [st[:, v:v + 1] for v in range(4)]
            s_f, s_b = st[:, 4:5], st[:, 5:6]

            def load_select(dst, consts, rows):
                """dst = sum_v onehot[v] * consts[v][rows], staging via pvar."""
                nr = rows[1] - rows[0]
                t = pvar.tile([nr, dst.shape[-1]], dst.dtype, tag="v", name="v")
                nc.sync.dma_start(out=t, in_=consts[0][rows[0]:rows[1], :])
                nc.vector.tensor_scalar(dst, t, st[0:nr, 0:1], None, op0=MULT)
                for v in range(1, 4):
                    t = pvar.tile([nr, dst.shape[-1]], dst.dtype, tag="v",
                                  name="v")
                    nc.sync.dma_start(out=t, in_=consts[v][rows[0]:rows[1], :])
                    nc.vector.scalar_tensor_tensor(
                        out=dst, in0=t, scalar=st[0:nr, v:v + 1], in1=dst,
                        op0=MULT, op1=ADD)

            xpw_all = pc.tile([128, NBLK * 96], BF, tag="xpw", name="xpw")
            load_select(xpw_all, xp_c, (0, 128))
            xpw = [xpw_all[:, D * 96:(D + 1) * 96] for D in range(NBLK)]
            dtw = pc.tile([DT_RANK, DL], BF, tag="dtw", name="dtw")
            load_select(dtw, dw_c, (0, DT_RANK))
            cst = pc.tile([128, 192], F32, tag="cst", name="cst")
            load_select(cst, cs_c, (0, 128))
            convw = cst[:, 0:32]
            convb = cst[:, 32:40]
            szb = cst[:, 40:48]
            dtb = cst[:, 48:56]
            acol = cst[:, 56:184]
            dcol = cst[:, 184:192]

            ones_m = pc.tile([128, 128], BF, tag="ones", name="ones")
            nc.vector.memset(ones_m, 1.0 / D_MODEL)
            epsb = pc.tile([128, 1], F32, tag="epsb", name="epsb")
            nc.vector.memset(epsb, 1e-5)

            # ---- phase 1: LN stats; xn = (x - mu) * rstd ----
            xn = []
            with tc.tile_pool(name="p1", bufs=1) as p1:
                raw = []
                for D in range(NBLK):
                    t = p1.tile([128, SEQ], BF, tag=f"raw{D}", name="raw")
                    nc.sync.dma_start(out=t, in_=xT[D * 128:(D + 1) * 128, :])
                    raw.append(t)
                mur = p1.tile([128, SEQ], F32, tag="mur", name="mur")
                rstd = p1.tile([128, SEQ], F32, tag="rstd", name="rstd")
                with tc.tile_pool(name="psA", bufs=1, space="PSUM") as psA:
                    mu_ps = psA.tile([128, SEQ], F32, tag="mu", name="mu")
                    ex2_ps = psA.tile([128, SEQ], F32, tag="ex2", name="ex2")
                    for D in range(NBLK):
                        xsq = p1.tile([128, SEQ], BF, tag="xsq", name="xsq",
                                      bufs=1)
                        nc.gpsimd.tensor_mul(xsq, raw[D], raw[D])
                        for c in range(NTC):
                            sl = bass.ts(c, 512)
                            nc.tensor.matmul(mu_ps[:, sl], ones_m[:],
                                             raw[D][:, sl],
                                             start=(D == 0), stop=(D == NBLK - 1))
                            nc.tensor.matmul(ex2_ps[:, sl], ones_m[:],
                                             xsq[:, sl],
                                             start=(D == 0), stop=(D == NBLK - 1))
                    nc.scalar.activation(mur, mu_ps[:], AF.Copy)
                    nc.vector.tensor_mul(rstd, mur, mur)
                    nc.vector.tensor_sub(rstd, ex2_ps[:], rstd)
                nc.scalar.activation(rstd, rstd, AF.Sqrt, bias=epsb[:, 0:1])
                nc.vector.reciprocal(rstd, rstd)

                for D in range(NBLK):
                    xs_t = pxn.tile([128, SEQ], BF, tag="xn", name="xn")
                    nc.vector.tensor_sub(xs_t, raw[D], mur)
                    nc.vector.tensor_mul(xs_t, xs_t, rstd)
                    xn.append(xs_t)

            # ---- phase 2: in_proj ----
            xr = []
            with tc.tile_pool(name="p2", bufs=1) as p2:
                winT = []
                for D in range(NBLK):
                    t = p2.tile([128, 2 * DL], BF, tag=f"wi{D}", name="wi")
                    load_select(t, wi_c, (D * 128, (D + 1) * 128))
                    winT.append(t)
                with tc.tile_pool(name="psB", bufs=4, space="PSUM") as psB:
                    for m in range(16):
                        if m < NBLK:
                            xt = pchain.tile([128, 3 + SEQ], BF, tag="chain",
                                             name="chain")
                            nc.vector.memset(xt[:, 0:3], 0.0)
                            xr.append(xt)
                        for cc in range(2):
                            pxz = psB.tile([128, 1024], F32, tag="ps", name="ps")
                            for half in range(2):
                                sl = bass.ts(2 * cc + half, 512)
                                for D in range(NBLK):
                                    nc.tensor.matmul(
                                        pxz[:, bass.ts(half, 512)],
                                        winT[D][:, bass.ts(m, 128)],
                                        xn[D][:, sl],
                                        start=(D == 0), stop=(D == NBLK - 1))
                            if m < NBLK:
                                nc.scalar.activation(
                                    xr[m][:, 3 + cc * 1024:3 + (cc + 1) * 1024],
                                    pxz[:], AF.Copy)
                            else:
                                gst = pmisc.tile([128, 1024], BF, tag="gst",
                                                 name="gst", bufs=2)
                                nc.scalar.activation(
                                    gst, pxz[:], AF.Silu,
                                    bias=szb[:, m - NBLK:m - NBLK + 1])
                                nc.sync.dma_start(
                                    out=gate_dram[(m - NBLK) * 128:
                                                  (m - NBLK + 1) * 128,
                                                  bass.ts(cc, 1024)],
                                    in_=gst)

            # ---- phase 3: causal depthwise conv + silu -> u ----
            u = []
            for D in range(NBLK):
                acc = pmisc.tile([128, SEQ], BF, tag="cacc", name="cacc", bufs=1)
                nc.vector.tensor_scalar(acc, xr[D][:, 0:SEQ],
                                        convw[:, 4 * D:4 * D + 1], None, op0=MULT)
                for k in range(1, D_CONV):
                    nc.vector.scalar_tensor_tensor(
                        out=acc, in0=xr[D][:, k:k + SEQ],
                        scalar=convw[:, 4 * D + k:4 * D + k + 1], in1=acc,
                        op0=MULT, op1=ADD)
                ut = pchain.tile([128, SEQ], BF, tag="chain", name="chain")
                nc.scalar.activation(ut, acc, AF.Silu, bias=convb[:, D:D + 1])
                u.append(ut)

            # ---- phases 4-9 ----
            with (
                tc.tile_pool(name="p5", bufs=1) as p5,
                tc.tile_pool(name="pxs", bufs=8) as pxs,      # dtu -> ysel
                tc.tile_pool(name="py", bufs=8) as py,        # yac
            ):
              with tc.tile_pool(name="psC", bufs=2, space="PSUM") as psC:
                # x_proj partial + pair AllReduce
                pdbc = psC.tile([128, SEQ], F32, tag="p4", name="p4")
                for c in range(NTC):
                    for D in range(NBLK):
                        nc.tensor.matmul(pdbc[0:96, bass.ts(c, 512)], xpw[D],
                                         u[D][:, bass.ts(c, 512)],
                                         start=(D == 0), stop=(D == NBLK - 1))
                dst = pmisc.tile([96, SEQ], BF, tag="dbcst", name="dbcst", bufs=1)
                nc.scalar.activation(dst, pdbc[0:96, :], AF.Copy)
                nc.sync.dma_start(out=cc_in[:], in_=dst)
                if sim:
                    nc.sync.dma_start(out=cc_out[:], in_=cc_in[:])
                else:
                    nc.gpsimd.collective_compute(
                        "AllReduce", ADD,
                        replica_groups=[[0, 1], [2, 3], [4, 5], [6, 7]],
                        ins=[cc_in[:]], outs=[cc_out[:]])
                # AR-independent work, overlaps the collective
                ebc = p5.tile([96, 32 * 128], BF, tag="ebc", name="ebc")
                nc.sync.dma_start(out=ebc, in_=ebc_c[:])
                yac = []
                for D in range(NBLK):
                    yt = py.tile([128, SEQ], BF, tag="y", name="y")
                    nc.vector.tensor_scalar(yt, u[D], dcol[:, D:D + 1], None,
                                            op0=MULT)
                    yac.append(yt)
                dbc = p5.tile([96, SEQ], BF, tag="dbc", name="dbc")
                nc.sync.dma_start(out=dbc, in_=cc_out[:])

                # dt = softplus(dtw@dbc + dtb) via exp series
                dt = []
                dtu = []
                for D in range(NBLK):
                    pdt = psC.tile([128, SEQ], F32, tag="p4", name="p4")
                    for c in range(NTC):
                        nc.tensor.matmul(pdt[:, bass.ts(c, 512)],
                                         dtw[:, bass.ts(D, 128)],
                                         dbc[0:DT_RANK, bass.ts(c, 512)],
                                         start=True, stop=True)
                    ex = pmisc.tile([128, SEQ], BF, tag="spx", name="spx", bufs=1)
                    nc.scalar.activation(ex, pdt[:], AF.Exp, bias=dtb[:, D:D + 1])
                    q = pmisc.tile([128, SEQ], BF, tag="q", name="q", bufs=1)
                    nc.vector.tensor_scalar(q, ex, -1.0 / 3.0, 0.5,
                                            op0=MULT, op1=ADD)
                    nc.vector.tensor_mul(q, ex, q)
                    nc.vector.tensor_scalar(q, q, -1.0, 1.0, op0=MULT, op1=ADD)
                    dtt = pchain.tile([128, SEQ], BF, tag="chain", name="chain")
                    nc.vector.tensor_mul(dtt, ex, q)
                    dt.append(dtt)
                    dut = pxs.tile([128, SEQ], BF, tag="xs", name="xs")
                    nc.vector.tensor_mul(dut, dtt, u[D])
                    dtu.append(dut)

                # selective scan
                with tc.tile_pool(name="pbc", bufs=2) as pbc:
                    for n in range(D_STATE):
                        pb = psC.tile([128, SEQ], F32, tag="p4", name="p4")
                        for c in range(NTC):
                            nc.tensor.matmul(pb[:, bass.ts(c, 512)],
                                             ebc[64:96, bass.ts(n, 128)],
                                             dbc[64:96, bass.ts(c, 512)],
                                             start=True, stop=True)
                        brep = pbc.tile([128, SEQ], BF, tag="brep", name="brep")
                        nc.scalar.activation(brep, pb[:], AF.Copy)
                        pcs = psC.tile([128, SEQ], F32, tag="p4", name="p4")
                        for c in range(NTC):
                            nc.tensor.matmul(pcs[:, bass.ts(c, 512)],
                                             ebc[64:96, bass.ts(16 + n, 128)],
                                             dbc[64:96, bass.ts(c, 512)],
                                             start=True, stop=True)
                        crep = pbc.tile([128, SEQ], BF, tag="crep", name="crep")
                        nc.scalar.activation(crep, pcs[:], AF.Copy)
                        for D in range(NBLK):
                            av = pxn.tile([128, SEQ], BF, tag="xn", name="xn")
                            nc.scalar.activation(
                                av, dt[D], AF.Exp,
                                scale=acol[:, D * D_STATE + n:
                                           D * D_STATE + n + 1])
                            bv = pxn.tile([128, SEQ], BF, tag="xn", name="xn")
                            nc.vector.tensor_mul(bv, dtu[D], brep)
                            nc.vector.tensor_tensor_scan(av, av, bv, 0.0,
                                                         op0=MULT, op1=ADD)
                            nc.vector.tensor_mul(bv, av, crep)
                            nc.gpsimd.tensor_add(yac[D], yac[D], bv)

              # ---- phase 8: gating ----
              for D in range(NBLK):
                    g = pgs.tile([128, SEQ], BF, tag="gs", name="gs")
                    nc.sync.dma_start(out=g,
                                      in_=gate_dram[D * 128:(D + 1) * 128, :])
                    nc.gpsimd.tensor_mul(yac[D], yac[D], g)
              ysel = yac

              # ---- phase 9: fused out_proj @ proj ----
              with (
                    tc.tile_pool(name="p9", bufs=1) as p9,
                    tc.tile_pool(name="psD", bufs=4, space="PSUM") as psD,
              ):
                    wf = []
                    for D in range(NBLK):
                        t = p9.tile([128, D_MODEL], BF, tag=f"wf{D}", name="wf")
                        load_select(t, wf_c, (D * 128, (D + 1) * 128))
                        wf.append(t)
                    for m in range(NMT):
                        po = psD.tile([128, 1024], F32, tag="po", name="po")
                        for oc in range(2):
                            for D in range(NBLK):
                                nc.tensor.matmul(po[:, bass.ts(oc, 512)],
                                                 ysel[D][:, bass.ts(m, 128)],
                                                 wf[D][:, bass.ts(oc, 512)],
                                                 start=(D == 0),
                                                 stop=(D == NBLK - 1))
                        ot = pgs.tile([128, 1024], BF, tag="gs", name="gs")
                        nc.scalar.activation(ot, po[:], AF.Copy)
                        nc.sync.dma_start(
                            out=y_part[m * 128:(m + 1) * 128, :],
                            in_=ot)
    nc.compile()
    return nc


_CACHE = {}


def _weights_key(inputs):
    hs = hashlib.sha1()
    for k in sorted(inputs):
        if k != "x":
            hs.update(np.ascontiguousarray(np.asarray(inputs[k])).tobytes())
    return hs.hexdigest()


def _get_runner(inputs):
    key = _weights_key(inputs)
    if _CACHE.get("key") == key:
        return _CACHE["runner"]
    import jax
    from jax.sharding import Mesh, PartitionSpec
    from jax.experimental.shard_map import shard_map
    from concourse import bass2jax

    nc = _build_program(inputs)
    bass2jax.install_neuronx_cc_hook()

    partition_name = nc.partition_id_tensor.name if nc.partition_id_tensor else None
    in_names, out_names, out_avals, zero_outs = [], [], [], []
    for alloc in nc.m.functions[0].allocations:
        if not isinstance(alloc, mybir.MemoryLocationSet):
            continue
        name = alloc.memorylocations[0].name
        if alloc.kind == "ExternalInput":
            if name != partition_name:
                in_names.append(name)
        elif alloc.kind == "ExternalOutput":
            out_names.append(name)
            shape = tuple(alloc.tensor_shape)
            dtype = mybir.dt.np(alloc.dtype)
            out_avals.append(jax.core.ShapedArray(shape, dtype))
            zero_outs.append(np.zeros(shape, dtype))
    n_params = len(in_names)
    all_in_names = list(in_names) + list(out_names)
    if partition_name is not None:
        all_in_names.append(partition_name)

    def _body(*args):
        operands = list(args)
        if partition_name is not None:
            operands.append(bass2jax.partition_id_tensor())
        outs = bass2jax._bass_exec_p.bind(
            *operands,
            out_avals=tuple(out_avals),
            in_names=tuple(all_in_names),
            out_names=tuple(out_names),
            lowering_input_output_aliases=(),
            sim_require_finite=True,
            sim_require_nnan=True,
            nc=nc,
        )
        return tuple(outs)

    devices = jax.devices()[:8]
    mesh = Mesh(np.asarray(devices), ("core",))
    n_outs = len(out_avals)
    in_specs = (PartitionSpec("core"),) * (n_params + n_outs)
    out_specs = (PartitionSpec("core"),) * n_outs
    sharded = jax.jit(
        shard_map(_body, mesh=mesh, in_specs=in_specs, out_specs=out_specs,
                  check_rep=False),
        keep_unused=True)

    def prepare(maps):
        per_core = [[np.asarray(m[nm]) for nm in in_names] for m in maps]
        concat_in = [np.concatenate([per_core[c][i] for c in range(8)], axis=0)
                     for i in range(n_params)]
        concat_zeros = [np.zeros((8 * z.shape[0], *z.shape[1:]), z.dtype)
                        for z in zero_outs]
        return concat_in + concat_zeros

    def call(args):
        return sharded(*args)

    def to_results(out_arrs):
        return [
            {nm: np.asarray(out_arrs[i]).reshape(8, *out_avals[i].shape)[c]
             for i, nm in enumerate(out_names)}
            for c in range(8)
        ]

    def runner(maps):
        return to_results(call(prepare(maps)))

    runner.prepare = prepare
    runner.call = call
    runner.to_results = to_results
    _CACHE["key"] = key
    _CACHE["runner"] = runner
    return runner


def make_in_maps(inputs):
    x = np.asarray(inputs["x"], np.float32)
    maps = []
    for c in range(8):
        b, r, h = c // 4, (c // 2) % 2, c % 2
        xb = x[b]
        if r == 1:
            xb = xb[::-1]
        xTv = np.ascontiguousarray(xb.T).astype(BF16)
        s = np.zeros((128, 8), np.float32)
        s[:, 2 * r + h] = 1.0
        s[:, 4] = 1.0 if r == 0 else 0.0
        s[:, 5] = 0.0 if r == 0 else 1.0
        maps.append({"xT": xTv, "sel": s})
    return maps


def gather(inputs, results):
    x = np.asarray(inputs["x"], np.float32)
    proj_b = np.asarray(inputs["proj_b"], np.float32)
    out = x + proj_b[None, None, :]
    for c in range(8):
        b, r = c // 4, (c // 2) % 2
        part = np.asarray(results[c]["y_part"]).astype(np.float32)
        if r == 1:
            part = part[::-1]
        out[b] += part
    return out


def kernel(**inputs) -> np.ndarray:
    runner = _get_runner(inputs)
    maps = make_in_maps(inputs)
    results = runner(maps)
    return gather(inputs, results)


# revision 41
# speedup vs baseline: 1.1965x; 1.1965x over previous
"""BiMambaBlock Trainium2 Bass kernel (low-I/O design).

Sharding: 8 cores = (batch b) x (branch r in {fwd,bwd}) x (d_inner half h).
All call-invariant data (weights, folded biases, A, D, one-hot helpers) is
baked into the NEFF as inline constants in 4 (r,h)-variants; each core picks
its variant on-device with a tiny one-hot selector input, so the only
per-call traffic is a 1MB x-quarter in (AllGathered on-device) and a 1MB
bf16 y-chunk out (ReduceScattered on-device).  The bwd branch's time flip is
done on-device with reversed-AP DVE copies blended by per-core 0/1 scalars
(SPMD-safe: LN stats are flip-equivariant, so xn is normalized first and
then flip-selected).  B/C scan coefficients are broadcast across partitions
with PE one-hot matmuls instead of stride-0 DMA.
"""

import os
import sys

for _p in ("/opt/trn_rl_repo", "/root/.axon_site/_ro/trn_rl_repo"):
    if os.path.isdir(_p) and _p not in sys.path:
        sys.path.insert(0, _p)
        break

import hashlib
import numpy as np
import ml_dtypes

import concourse.bass as bass
import concourse.mybir as mybir
import concourse.tile as tile
from concourse import bacc

BF16 = ml_dtypes.bfloat16
F32 = mybir.dt.float32
BF = mybir.dt.bfloat16

D_MODEL = 1024
D_INNER = 2048
D_STATE = 16
D_CONV = 4
DT_RANK = 64
BATCH, SEQ = 2, 2048
DL = 1024          # local d_inner half per core
NBLK = DL // 128   # 8 d-blocks of 128
NTC = SEQ // 512   # 4 chunks of 512
NMT = SEQ // 128   # 16 time tiles of 128

MULT = mybir.AluOpType.mult
ADD = mybir.AluOpType.add
SUB = mybir.AluOpType.subtract
AF = mybir.ActivationFunctionType


def _rev(ap):
    """Free-dim reversed view of a 2D AP."""
    n = ap.ap[-1][1]
    return bass.AP(tensor=ap.tensor, offset=ap.offset + (n - 1) * ap.ap[-1][0],
                   ap=[list(ap.ap[0]), [-ap.ap[-1][0], n]])


def _variant_consts(inputs):
    """Host: fold weights into the 4 (r,h) inline-const variants."""
    ln_g = np.asarray(inputs["ln_g"], np.float32)
    ln_b = np.asarray(inputs["ln_b"], np.float32)
    proj_w = np.asarray(inputs["proj_w"], np.float32)
    out = []
    for v in range(4):
        r, h = v // 2, v % 2
        p = "fwd" if r == 0 else "bwd"
        in_w = np.asarray(inputs[p + "_in_w"], np.float32)
        conv_w = np.asarray(inputs[p + "_conv_w"], np.float32)
        conv_b = np.asarray(inputs[p + "_conv_b"], np.float32)
        xproj_w = np.asarray(inputs[p + "_xproj_w"], np.float32)
        dt_w = np.asarray(inputs[p + "_dt_w"], np.float32)
        dt_b = np.asarray(inputs[p + "_dt_b"], np.float32)
        A_log = np.asarray(inputs[p + "_A_log"], np.float32)
        Dp = np.asarray(inputs[p + "_D"], np.float32)
        out_w = np.asarray(inputs[p + "_out_w"], np.float32)

        sl = slice(h * DL, (h + 1) * DL)
        W = np.concatenate(
            [in_w[sl], in_w[D_INNER + h * DL:D_INNER + (h + 1) * DL]], 0)
        W = W * ln_g[None, :]
        cb = W @ ln_b
        cb_x, cb_z = cb[:DL], cb[DL:]
        w_inT = np.ascontiguousarray(W.T).astype(BF16)          # [1024, 2048]

        cwl = conv_w[sl]
        conv_b_eff = conv_b[sl] + cb_x * cwl.sum(1)
        conv_w_c = np.ascontiguousarray(
            cwl.reshape(NBLK, 128, D_CONV).transpose(1, 0, 2)
            .reshape(128, NBLK * D_CONV)).astype(np.float32)

        def col(vv):
            return np.ascontiguousarray(vv.reshape(NBLK, 128).T).astype(np.float32)

        A = -np.exp(A_log[sl])
        a_cols = np.ascontiguousarray(
            A.reshape(NBLK, 128, D_STATE).transpose(1, 0, 2)
            .reshape(128, NBLK * D_STATE)).astype(np.float32)

        w_fold = proj_w[:, r * D_MODEL:(r + 1) * D_MODEL] @ out_w[:, sl]
        w_foldT = np.ascontiguousarray(w_fold.T).astype(BF16)   # [1024, 1024]

        xpw = np.ascontiguousarray(xproj_w[:, sl].T).astype(BF16)  # [1024, 96]
        xpw_pack = np.ascontiguousarray(
            xpw.reshape(NBLK, 128, 96).transpose(1, 0, 2)
            .reshape(128, NBLK * 96)).astype(BF16)              # [128, 768]
        dt_wT = np.ascontiguousarray(dt_w[sl].T).astype(BF16)   # [64, 1024]

        cst = np.concatenate([
            conv_w_c,                 # 0:32
            col(conv_b_eff),          # 32:40
            col(cb_z),                # 40:48
            col(dt_b[sl]),            # 48:56
            a_cols,                   # 56:184
            col(Dp[sl]),              # 184:192
        ], axis=1).astype(np.float32)                           # [128, 192]

        out.append({
            "wi": w_inT, "wf": w_foldT, "xp": xpw_pack,
            "dw": dt_wT, "cst": cst,
        })
    return out


def _build_program(inputs, sim=False):
    var = _variant_consts(inputs)

    nc = bacc.Bacc("TRN2", target_bir_lowering=False, debug=False, num_devices=8)

    # ---- per-core external I/O ----
    xT = nc.declare_dram_parameter("xT", [D_MODEL, SEQ], BF, isOutput=False)
    sel = nc.declare_dram_parameter("sel", [128, 8], F32, isOutput=False)
    y_part = nc.declare_dram_parameter("y_part", [512, D_MODEL], BF, isOutput=True)

    # ---- inline consts ----
    wi_c = [nc.inline_tensor(var[v]["wi"], name=f"wi{v}") for v in range(4)]
    wf_c = [nc.inline_tensor(var[v]["wf"], name=f"wf{v}") for v in range(4)]
    xp_c = [nc.inline_tensor(var[v]["xp"], name=f"xp{v}") for v in range(4)]
    dw_c = [nc.inline_tensor(var[v]["dw"], name=f"dw{v}") for v in range(4)]
    cs_c = [nc.inline_tensor(var[v]["cst"], name=f"cs{v}") for v in range(4)]
    ebc_np = np.zeros((96, 32 * 128), np.float32)
    for k in range(32):
        ebc_np[64 + k, k * 128:(k + 1) * 128] = 1.0
    ebc_c = nc.inline_tensor(ebc_np.astype(BF16), name="ebc")

    # ---- internal DRAM ----
    cc_in = nc.dram_tensor("cc_in", [96, SEQ], BF)
    cc_out = nc.dram_tensor("cc_out", [96, SEQ], BF)
    gate_dram = nc.dram_tensor("gate_dram", [DL, SEQ], BF)
    yrs_in = nc.dram_tensor("yrs_in", [SEQ, D_MODEL], BF)
    yrs_out = nc.dram_tensor("yrs_out", [512, D_MODEL], BF)

    with tile.TileContext(nc) as tc:
        with (
            tc.tile_pool(name="pc", bufs=1) as pc,            # small consts
            tc.tile_pool(name="pvar", bufs=2) as pvar,        # variant staging
            tc.tile_pool(name="pxn", bufs=8) as pxn,          # xn -> av/bv
            tc.tile_pool(name="pchain", bufs=9) as pchain,    # xr -> u -> dt
            tc.tile_pool(name="pmisc", bufs=2) as pmisc,
            tc.tile_pool(name="pgs", bufs=3) as pgs,          # small streams
        ):
            st = pc.tile([128, 8], F32, tag="st", name="st")
            nc.sync.dma_start(out=st, in_=sel[:])
            s_v = [st[:, v:v + 1] for v in range(4)]
            s_f, s_b = st[:, 4:5], st[:, 5:6]

            def load_select(dst, consts, rows):
                """dst = sum_v onehot[v] * consts[v][rows], staging via pvar."""
                nr = rows[1] - rows[0]
                t = pvar.tile([nr, dst.shape[-1]], dst.dtype, tag="v", name="v")
                nc.sync.dma_start(out=t, in_=consts[0][rows[0]:rows[1], :])
                nc.vector.tensor_scalar(dst, t, st[0:nr, 0:1], None, op0=MULT)
                for v in range(1, 4):
                    t = pvar.tile([nr, dst.shape[-1]], dst.dtype, tag="v",
                                  name="v")
                    nc.sync.dma_start(out=t, in_=consts[v][rows[0]:rows[1], :])
                    nc.vector.scalar_tensor_tensor(
                        out=dst, in0=t, scalar=st[0:nr, v:v + 1], in1=dst,
                        op0=MULT, op1=ADD)

            xpw_all = pc.tile([128, NBLK * 96], BF, tag="xpw", name="xpw")
            load_select(xpw_all, xp_c, (0, 128))
            xpw = [xpw_all[:, D * 96:(D + 1) * 96] for D in range(NBLK)]
            dtw = pc.tile([DT_RANK, DL], BF, tag="dtw", name="dtw")
            load_select(dtw, dw_c, (0, DT_RANK))
            cst = pc.tile([128, 192], F32, tag="cst", name="cst")
            load_select(cst, cs_c, (0, 128))
            convw = cst[:, 0:32]
            convb = cst[:, 32:40]
            szb = cst[:, 40:48]
            dtb = cst[:, 48:56]
            acol = cst[:, 56:184]
            dcol = cst[:, 184:192]

            ones_m = pc.tile([128, 128], BF, tag="ones", name="ones")
            nc.vector.memset(ones_m, 1.0 / D_MODEL)
            epsb = pc.tile([128, 1], F32, tag="epsb", name="epsb")
            nc.vector.memset(epsb, 1e-5)

            # ---- phase 1: LN stats; xn = (x - mu) * rstd ----
            xn = []
            with tc.tile_pool(name="p1", bufs=1) as p1:
                raw = []
                for D in range(NBLK):
                    t = p1.tile([128, SEQ], BF, tag=f"raw{D}", name="raw")
                    nc.sync.dma_start(out=t, in_=xT[D * 128:(D + 1) * 128, :])
                    raw.append(t)
                mur = p1.tile([128, SEQ], F32, tag="mur", name="mur")
                rstd = p1.tile([128, SEQ], F32, tag="rstd", name="rstd")
                with tc.tile_pool(name="psA", bufs=1, space="PSUM") as psA:
                    mu_ps = psA.tile([128, SEQ], F32, tag="mu", name="mu")
                    ex2_ps = psA.tile([128, SEQ], F32, tag="ex2", name="ex2")
                    for D in range(NBLK):
                        xsq = p1.tile([128, SEQ], BF, tag="xsq", name="xsq",
                                      bufs=1)
                        nc.gpsimd.tensor_mul(xsq, raw[D], raw[D])
                        for c in range(NTC):
                            sl = bass.ts(c, 512)
                            nc.tensor.matmul(mu_ps[:, sl], ones_m[:],
                                             raw[D][:, sl],
                                             start=(D == 0), stop=(D == NBLK - 1))
                            nc.tensor.matmul(ex2_ps[:, sl], ones_m[:],
                                             xsq[:, sl],
                                             start=(D == 0), stop=(D == NBLK - 1))
                    nc.scalar.activation(mur, mu_ps[:], AF.Copy)
                    nc.vector.tensor_mul(rstd, mur, mur)
                    nc.vector.tensor_sub(rstd, ex2_ps[:], rstd)
                nc.scalar.activation(rstd, rstd, AF.Sqrt, bias=epsb[:, 0:1])
                nc.vector.reciprocal(rstd, rstd)

                for D in range(NBLK):
                    xs_t = pxn.tile([128, SEQ], BF, tag="xn", name="xn")
                    nc.vector.tensor_sub(xs_t, raw[D], mur)
                    nc.vector.tensor_mul(xs_t, xs_t, rstd)
                    xn.append(xs_t)

            # ---- phase 2: in_proj ----
            xr = []
            with tc.tile_pool(name="p2", bufs=1) as p2:
                winT = []
                for D in range(NBLK):
                    t = p2.tile([128, 2 * DL], BF, tag=f"wi{D}", name="wi")
                    load_select(t, wi_c, (D * 128, (D + 1) * 128))
                    winT.append(t)
                with tc.tile_pool(name="psB", bufs=4, space="PSUM") as psB:
                    for m in range(16):
                        if m < NBLK:
                            xt = pchain.tile([128, 3 + SEQ], BF, tag="chain",
                                             name="chain")
                            nc.vector.memset(xt[:, 0:3], 0.0)
                            xr.append(xt)
                        for cc in range(2):
                            pxz = psB.tile([128, 1024], F32, tag="ps", name="ps")
                            for half in range(2):
                                sl = bass.ts(2 * cc + half, 512)
                                for D in range(NBLK):
                                    nc.tensor.matmul(
                                        pxz[:, bass.ts(half, 512)],
                                        winT[D][:, bass.ts(m, 128)],
                                        xn[D][:, sl],
                                        start=(D == 0), stop=(D == NBLK - 1))
                            if m < NBLK:
                                nc.scalar.activation(
                                    xr[m][:, 3 + cc * 1024:3 + (cc + 1) * 1024],
                                    pxz[:], AF.Copy)
                            else:
                                gst = pmisc.tile([128, 1024], BF, tag="gst",
                                                 name="gst", bufs=2)
                                nc.scalar.activation(
                                    gst, pxz[:], AF.Silu,
                                    bias=szb[:, m - NBLK:m - NBLK + 1])
                                nc.sync.dma_start(
                                    out=gate_dram[(m - NBLK) * 128:
                                                  (m - NBLK + 1) * 128,
                                                  bass.ts(cc, 1024)],
                                    in_=gst)

            # ---- phase 3: causal depthwise conv + silu -> u ----
            u = []
            for D in range(NBLK):
                acc = pmisc.tile([128, SEQ], BF, tag="cacc", name="cacc", bufs=1)
                nc.vector.tensor_scalar(acc, xr[D][:, 0:SEQ],
                                        convw[:, 4 * D:4 * D + 1], None, op0=MULT)
                for k in range(1, D_CONV):
                    nc.vector.scalar_tensor_tensor(
                        out=acc, in0=xr[D][:, k:k + SEQ],
                        scalar=convw[:, 4 * D + k:4 * D + k + 1], in1=acc,
                        op0=MULT, op1=ADD)
                ut = pchain.tile([128, SEQ], BF, tag="chain", name="chain")
                nc.scalar.activation(ut, acc, AF.Silu, bias=convb[:, D:D + 1])
                u.append(ut)

            # ---- phases 4-9 ----
            with (
                tc.tile_pool(name="p5", bufs=1) as p5,
                tc.tile_pool(name="pxs", bufs=8) as pxs,      # dtu -> ysel
                tc.tile_pool(name="py", bufs=8) as py,        # yac
            ):
              with tc.tile_pool(name="psC", bufs=2, space="PSUM") as psC:
                # x_proj partial + pair AllReduce
                pdbc = psC.tile([128, SEQ], F32, tag="p4", name="p4")
                for c in range(NTC):
                    for D in range(NBLK):
                        nc.tensor.matmul(pdbc[0:96, bass.ts(c, 512)], xpw[D],
                                         u[D][:, bass.ts(c, 512)],
                                         start=(D == 0), stop=(D == NBLK - 1))
                dst = pmisc.tile([96, SEQ], BF, tag="dbcst", name="dbcst", bufs=1)
                nc.scalar.activation(dst, pdbc[0:96, :], AF.Copy)
                nc.sync.dma_start(out=cc_in[:], in_=dst)
                if sim:
                    nc.sync.dma_start(out=cc_out[:], in_=cc_in[:])
                else:
                    nc.gpsimd.collective_compute(
                        "AllReduce", ADD,
                        replica_groups=[[0, 1], [2, 3], [4, 5], [6, 7]],
                        ins=[cc_in[:]], outs=[cc_out[:]])
                # AR-independent work, overlaps the collective
                ebc = p5.tile([96, 32 * 128], BF, tag="ebc", name="ebc")
                nc.sync.dma_start(out=ebc, in_=ebc_c[:])
                yac = []
                for D in range(NBLK):
                    yt = py.tile([128, SEQ], BF, tag="y", name="y")
                    nc.vector.tensor_scalar(yt, u[D], dcol[:, D:D + 1], None,
                                            op0=MULT)
                    yac.append(yt)
                dbc = p5.tile([96, SEQ], BF, tag="dbc", name="dbc")
                nc.sync.dma_start(out=dbc, in_=cc_out[:])

                # dt = softplus(dtw@dbc + dtb) via exp series
                dt = []
                dtu = []
                for D in range(NBLK):
                    pdt = psC.tile([128, SEQ], F32, tag="p4", name="p4")
                    for c in range(NTC):
                        nc.tensor.matmul(pdt[:, bass.ts(c, 512)],
                                         dtw[:, bass.ts(D, 128)],
                                         dbc[0:DT_RANK, bass.ts(c, 512)],
                                         start=True, stop=True)
                    ex = pmisc.tile([128, SEQ], BF, tag="spx", name="spx", bufs=1)
                    nc.scalar.activation(ex, pdt[:], AF.Exp, bias=dtb[:, D:D + 1])
                    q = pmisc.tile([128, SEQ], BF, tag="q", name="q", bufs=1)
                    nc.vector.tensor_scalar(q, ex, -1.0 / 3.0, 0.5,
                                            op0=MULT, op1=ADD)
                    nc.vector.tensor_mul(q, ex, q)
                    nc.vector.tensor_scalar(q, q, -1.0, 1.0, op0=MULT, op1=ADD)
                    dtt = pchain.tile([128, SEQ], BF, tag="chain", name="chain")
                    nc.vector.tensor_mul(dtt, ex, q)
                    dt.append(dtt)
                    dut = pxs.tile([128, SEQ], BF, tag="xs", name="xs")
                    nc.vector.tensor_mul(dut, dtt, u[D])
                    dtu.append(dut)

                # selective scan
                with tc.tile_pool(name="pbc", bufs=2) as pbc:
                    for n in range(D_STATE):
                        pb = psC.tile([128, SEQ], F32, tag="p4", name="p4")
                        for c in range(NTC):
                            nc.tensor.matmul(pb[:, bass.ts(c, 512)],
                                             ebc[64:96, bass.ts(n, 128)],
                                             dbc[64:96, bass.ts(c, 512)],
                                             start=True, stop=True)
                        brep = pbc.tile([128, SEQ], BF, tag="brep", name="brep")
                        nc.scalar.activation(brep, pb[:], AF.Copy)
                        pcs = psC.tile([128, SEQ], F32, tag="p4", name="p4")
                        for c in range(NTC):
                            nc.tensor.matmul(pcs[:, bass.ts(c, 512)],
                                             ebc[64:96, bass.ts(16 + n, 128)],
                                             dbc[64:96, bass.ts(c, 512)],
                                             start=True, stop=True)
                        crep = pbc.tile([128, SEQ], BF, tag="crep", name="crep")
                        nc.scalar.activation(crep, pcs[:], AF.Copy)
                        for D in range(NBLK):
                            av = pxn.tile([128, SEQ], BF, tag="xn", name="xn")
                            nc.scalar.activation(
                                av, dt[D], AF.Exp,
                                scale=acol[:, D * D_STATE + n:
                                           D * D_STATE + n + 1])
                            bv = pxn.tile([128, SEQ], BF, tag="xn", name="xn")
                            nc.vector.tensor_mul(bv, dtu[D], brep)
                            nc.vector.tensor_tensor_scan(av, av, bv, 0.0,
                                                         op0=MULT, op1=ADD)
                            nc.vector.tensor_mul(bv, av, crep)
                            nc.gpsimd.tensor_add(yac[D], yac[D], bv)

              # ---- phase 8: gating + flip-select back to true time ----
              ysel = []
              for D in range(NBLK):
                    g = pgs.tile([128, SEQ], BF, tag="gs", name="gs")
                    nc.sync.dma_start(out=g,
                                      in_=gate_dram[D * 128:(D + 1) * 128, :])
                    nc.gpsimd.tensor_mul(yac[D], yac[D], g)
                    ys = pxs.tile([128, SEQ], BF, tag="xs", name="xs")
                    nc.vector.tensor_scalar(ys, _rev(yac[D][:, :]), s_b, None,
                                            op0=MULT)
                    nc.vector.scalar_tensor_tensor(
                        out=ys, in0=yac[D], scalar=s_f, in1=ys,
                        op0=MULT, op1=ADD)
                    ysel.append(ys)

              # ---- phase 9: fused out_proj @ proj ----
              with (
                    tc.tile_pool(name="p9", bufs=1) as p9,
                    tc.tile_pool(name="psD", bufs=4, space="PSUM") as psD,
              ):
                    wf = []
                    for D in range(NBLK):
                        t = p9.tile([128, D_MODEL], BF, tag=f"wf{D}", name="wf")
                        load_select(t, wf_c, (D * 128, (D + 1) * 128))
                        wf.append(t)
                    for m in range(NMT):
                        po = psD.tile([128, 1024], F32, tag="po", name="po")
                        for oc in range(2):
                            for D in range(NBLK):
                                nc.tensor.matmul(po[:, bass.ts(oc, 512)],
                                                 ysel[D][:, bass.ts(m, 128)],
                                                 wf[D][:, bass.ts(oc, 512)],
                                                 start=(D == 0),
                                                 stop=(D == NBLK - 1))
                        ot = pgs.tile([128, 1024], BF, tag="gs", name="gs")
                        nc.scalar.activation(ot, po[:], AF.Copy)
                        nc.sync.dma_start(
                            out=yrs_in[m * 128:(m + 1) * 128, :],
                            in_=ot)

            # ---- ReduceScatter y over each batch group ----
            if sim:
                nc.sync.dma_start(out=yrs_out[:], in_=yrs_in[0:512, :])
            else:
                nc.gpsimd.collective_compute(
                    "ReduceScatter", ADD,
                    replica_groups=[[0, 1, 2, 3], [4, 5, 6, 7]],
                    ins=[yrs_in[:]], outs=[yrs_out[:]])
            nc.sync.dma_start(out=y_part[:], in_=yrs_out[:])
    nc.compile()
    return nc


_CACHE = {}


def _weights_key(inputs):
    hs = hashlib.sha1()
    for k in sorted(inputs):
        if k != "x":
            hs.update(np.ascontiguousarray(np.asarray(inputs[k])).tobytes())
    return hs.hexdigest()


def _get_runner(inputs):
    key = _weights_key(inputs)
    if _CACHE.get("key") == key:
        return _CACHE["runner"]
    import jax
    from jax.sharding import Mesh, PartitionSpec
    from jax.experimental.shard_map import shard_map
    from concourse import bass2jax

    nc = _build_program(inputs)
    bass2jax.install_neuronx_cc_hook()

    partition_name = nc.partition_id_tensor.name if nc.partition_id_tensor else None
    in_names, out_names, out_avals, zero_outs = [], [], [], []
    for alloc in nc.m.functions[0].allocations:
        if not isinstance(alloc, mybir.MemoryLocationSet):
            continue
        name = alloc.memorylocations[0].name
        if alloc.kind == "ExternalInput":
            if name != partition_name:
                in_names.append(name)
        elif alloc.kind == "ExternalOutput":
            out_names.append(name)
            shape = tuple(alloc.tensor_shape)
            dtype = mybir.dt.np(alloc.dtype)
            out_avals.append(jax.core.ShapedArray(shape, dtype))
            zero_outs.append(np.zeros(shape, dtype))
    n_params = len(in_names)
    all_in_names = list(in_names) + list(out_names)
    if partition_name is not None:
        all_in_names.append(partition_name)

    def _body(*args):
        operands = list(args)
        if partition_name is not None:
            operands.append(bass2jax.partition_id_tensor())
        outs = bass2jax._bass_exec_p.bind(
            *operands,
            out_avals=tuple(out_avals),
            in_names=tuple(all_in_names),
            out_names=tuple(out_names),
            lowering_input_output_aliases=(),
            sim_require_finite=True,
            sim_require_nnan=True,
            nc=nc,
        )
        return tuple(outs)

    devices = jax.devices()[:8]
    mesh = Mesh(np.asarray(devices), ("core",))
    n_outs = len(out_avals)
    in_specs = (PartitionSpec("core"),) * (n_params + n_outs)
    out_specs = (PartitionSpec("core"),) * n_outs
    sharded = jax.jit(
        shard_map(_body, mesh=mesh, in_specs=in_specs, out_specs=out_specs,
                  check_rep=False),
        keep_unused=True)

    def prepare(maps):
        per_core = [[np.asarray(m[nm]) for nm in in_names] for m in maps]
        concat_in = [np.concatenate([per_core[c][i] for c in range(8)], axis=0)
                     for i in range(n_params)]
        concat_zeros = [np.zeros((8 * z.shape[0], *z.shape[1:]), z.dtype)
                        for z in zero_outs]
        return concat_in + concat_zeros

    def call(args):
        return sharded(*args)

    def to_results(out_arrs):
        return [
            {nm: np.asarray(out_arrs[i]).reshape(8, *out_avals[i].shape)[c]
             for i, nm in enumerate(out_names)}
            for c in range(8)
        ]

    def runner(maps):
        return to_results(call(prepare(maps)))

    runner.prepare = prepare
    runner.call = call
    runner.to_results = to_results
    _CACHE["key"] = key
    _CACHE["runner"] = runner
    return runner


def make_in_maps(inputs):
    x = np.asarray(inputs["x"], np.float32)
    maps = []
    for c in range(8):
        b, r, h = c // 4, (c // 2) % 2, c % 2
        xb = x[b]
        if r == 1:
            xb = xb[::-1]
        xTv = np.ascontiguousarray(xb.T).astype(BF16)
        s = np.zeros((128, 8), np.float32)
        s[:, 2 * r + h] = 1.0
        s[:, 4] = 1.0 if r == 0 else 0.0
        s[:, 5] = 0.0 if r == 0 else 1.0
        maps.append({"xT": xTv, "sel": s})
    return maps


def gather(inputs, results):
    x = np.asarray(inputs["x"], np.float32)
    proj_b = np.asarray(inputs["proj_b"], np.float32)
    out = x + proj_b[None, None, :]
    for c in range(8):
        b, p = c // 4, c % 4
        part = np.asarray(results[c]["y_part"]).astype(np.float32)
        out[b, p * 512:(p + 1) * 512, :] += part
    return out


def kernel(**inputs) -> np.ndarray:
    runner = _get_runner(inputs)
    maps = make_in_maps(inputs)
    results = runner(maps)
    return gather(inputs, results)


# revision 46
# speedup vs baseline: 1.2068x; 1.0086x over previous
"""BiMambaBlock Trainium2 Bass kernel (low-I/O design).

Sharding: 8 cores = (batch b) x (branch r in {fwd,bwd}) x (d_inner half h).
All call-invariant data (weights, folded biases, A, D, one-hot helpers) is
baked into the NEFF as inline constants in 4 (r,h)-variants; each core picks
its variant on-device with a tiny one-hot selector input, so the only
per-call traffic is a 1MB x-quarter in (AllGathered on-device) and a 1MB
bf16 y-chunk out (ReduceScattered on-device).  The bwd branch's time flip is
done on-device with reversed-AP DVE copies blended by per-core 0/1 scalars
(SPMD-safe: LN stats are flip-equivariant, so xn is normalized first and
then flip-selected).  B/C scan coefficients are broadcast across partitions
with PE one-hot matmuls instead of stride-0 DMA.
"""

import os
import sys

for _p in ("/opt/trn_rl_repo", "/root/.axon_site/_ro/trn_rl_repo"):
    if os.path.isdir(_p) and _p not in sys.path:
        sys.path.insert(0, _p)
        break

import hashlib
import numpy as np
import ml_dtypes

import concourse.bass as bass
import concourse.mybir as mybir
import concourse.tile as tile
from concourse import bacc

BF16 = ml_dtypes.bfloat16
F32 = mybir.dt.float32
BF = mybir.dt.bfloat16

D_MODEL = 1024
D_INNER = 2048
D_STATE = 16
D_CONV = 4
DT_RANK = 64
BATCH, SEQ = 2, 2048
DL = 1024          # local d_inner half per core
NBLK = DL // 128   # 8 d-blocks of 128
NTC = SEQ // 512   # 4 chunks of 512
NMT = SEQ // 128   # 16 time tiles of 128

MULT = mybir.AluOpType.mult
ADD = mybir.AluOpType.add
SUB = mybir.AluOpType.subtract
AF = mybir.ActivationFunctionType


def _rev(ap):
    """Free-dim reversed view of a 2D AP."""
    n = ap.ap[-1][1]
    return bass.AP(tensor=ap.tensor, offset=ap.offset + (n - 1) * ap.ap[-1][0],
                   ap=[list(ap.ap[0]), [-ap.ap[-1][0], n]])


def _variant_consts(inputs):
    """Host: fold weights into the 4 (r,h) inline-const variants."""
    ln_g = np.asarray(inputs["ln_g"], np.float32)
    ln_b = np.asarray(inputs["ln_b"], np.float32)
    proj_w = np.asarray(inputs["proj_w"], np.float32)
    out = []
    for v in range(4):
        r, h = v // 2, v % 2
        p = "fwd" if r == 0 else "bwd"
        in_w = np.asarray(inputs[p + "_in_w"], np.float32)
        conv_w = np.asarray(inputs[p + "_conv_w"], np.float32)
        conv_b = np.asarray(inputs[p + "_conv_b"], np.float32)
        xproj_w = np.asarray(inputs[p + "_xproj_w"], np.float32)
        dt_w = np.asarray(inputs[p + "_dt_w"], np.float32)
        dt_b = np.asarray(inputs[p + "_dt_b"], np.float32)
        A_log = np.asarray(inputs[p + "_A_log"], np.float32)
        Dp = np.asarray(inputs[p + "_D"], np.float32)
        out_w = np.asarray(inputs[p + "_out_w"], np.float32)

        sl = slice(h * DL, (h + 1) * DL)
        W = np.concatenate(
            [in_w[sl], in_w[D_INNER + h * DL:D_INNER + (h + 1) * DL]], 0)
        W = W * ln_g[None, :]
        cb = W @ ln_b
        cb_x, cb_z = cb[:DL], cb[DL:]
        w_inT = np.ascontiguousarray(W.T).astype(BF16)          # [1024, 2048]

        cwl = conv_w[sl]
        conv_b_eff = conv_b[sl] + cb_x * cwl.sum(1)
        conv_w_c = np.ascontiguousarray(
            cwl.reshape(NBLK, 128, D_CONV).transpose(1, 0, 2)
            .reshape(128, NBLK * D_CONV)).astype(np.float32)

        def col(vv):
            return np.ascontiguousarray(vv.reshape(NBLK, 128).T).astype(np.float32)

        A = -np.exp(A_log[sl])
        a_cols = np.ascontiguousarray(
            A.reshape(NBLK, 128, D_STATE).transpose(1, 0, 2)
            .reshape(128, NBLK * D_STATE)).astype(np.float32)

        w_fold = proj_w[:, r * D_MODEL:(r + 1) * D_MODEL] @ out_w[:, sl]
        w_foldT = np.ascontiguousarray(w_fold.T).astype(BF16)   # [1024, 1024]

        xpw = np.ascontiguousarray(xproj_w[:, sl].T).astype(BF16)  # [1024, 96]
        xpw_pack = np.ascontiguousarray(
            xpw.reshape(NBLK, 128, 96).transpose(1, 0, 2)
            .reshape(128, NBLK * 96)).astype(BF16)              # [128, 768]
        dt_wT = np.ascontiguousarray(dt_w[sl].T).astype(BF16)   # [64, 1024]

        cst = np.concatenate([
            conv_w_c,                 # 0:32
            col(conv_b_eff),          # 32:40
            col(cb_z),                # 40:48
            col(dt_b[sl]),            # 48:56
            a_cols,                   # 56:184
            col(Dp[sl]),              # 184:192
        ], axis=1).astype(np.float32)                           # [128, 192]

        out.append({
            "wi": w_inT, "wf": w_foldT, "xp": xpw_pack,
            "dw": dt_wT, "cst": cst,
        })
    return out


def _build_program(inputs, sim=False):
    var = _variant_consts(inputs)

    nc = bacc.Bacc("TRN2", target_bir_lowering=False, debug=False, num_devices=8)

    # ---- per-core external I/O ----
    xq = nc.declare_dram_parameter("xq", [D_MODEL, 512], BF, isOutput=False)
    sel = nc.declare_dram_parameter("sel", [128, 8], F32, isOutput=False)
    y_part = nc.declare_dram_parameter("y_part", [512, D_MODEL], BF, isOutput=True)

    # ---- inline consts ----
    wi_c = [nc.inline_tensor(var[v]["wi"], name=f"wi{v}") for v in range(4)]
    wf_c = [nc.inline_tensor(var[v]["wf"], name=f"wf{v}") for v in range(4)]
    xp_c = [nc.inline_tensor(var[v]["xp"], name=f"xp{v}") for v in range(4)]
    dw_c = [nc.inline_tensor(var[v]["dw"], name=f"dw{v}") for v in range(4)]
    cs_c = [nc.inline_tensor(var[v]["cst"], name=f"cs{v}") for v in range(4)]
    ebc_np = np.zeros((96, 32 * 128), np.float32)
    for k in range(32):
        ebc_np[64 + k, k * 128:(k + 1) * 128] = 1.0
    ebc_c = nc.inline_tensor(ebc_np.astype(BF16), name="ebc")

    # ---- internal DRAM ----
    xg_src = nc.dram_tensor("xg_src", [D_MODEL, 512], BF)
    xg = nc.dram_tensor("xg", [4 * D_MODEL, 512], BF)
    cc_in = nc.dram_tensor("cc_in", [96, SEQ], BF)
    cc_out = nc.dram_tensor("cc_out", [96, SEQ], BF)
    gate_dram = nc.dram_tensor("gate_dram", [DL, SEQ], BF)
    yrs_in = nc.dram_tensor("yrs_in", [SEQ, D_MODEL], BF)
    yrs_out = nc.dram_tensor("yrs_out", [512, D_MODEL], BF)

    with tile.TileContext(nc) as tc:
        with (
            tc.tile_pool(name="pc", bufs=1) as pc,            # small consts
            tc.tile_pool(name="pvar", bufs=2) as pvar,        # variant staging
            tc.tile_pool(name="pxn", bufs=8) as pxn,          # xn -> av/bv
            tc.tile_pool(name="pchain", bufs=9) as pchain,    # xr -> u -> dt
            tc.tile_pool(name="pmisc", bufs=2) as pmisc,
            tc.tile_pool(name="pgs", bufs=3) as pgs,          # small streams
        ):
            st = pc.tile([128, 8], F32, tag="st", name="st")
            nc.sync.dma_start(out=st, in_=sel[:])
            s_v = [st[:, v:v + 1] for v in range(4)]
            s_f, s_b = st[:, 4:5], st[:, 5:6]

            def load_select(dst, consts, rows):
                """dst = sum_v onehot[v] * consts[v][rows], staging via pvar."""
                nr = rows[1] - rows[0]
                t = pvar.tile([nr, dst.shape[-1]], dst.dtype, tag="v", name="v")
                nc.sync.dma_start(out=t, in_=consts[0][rows[0]:rows[1], :])
                nc.vector.tensor_scalar(dst, t, st[0:nr, 0:1], None, op0=MULT)
                for v in range(1, 4):
                    t = pvar.tile([nr, dst.shape[-1]], dst.dtype, tag="v",
                                  name="v")
                    nc.sync.dma_start(out=t, in_=consts[v][rows[0]:rows[1], :])
                    nc.vector.scalar_tensor_tensor(
                        out=dst, in0=t, scalar=st[0:nr, v:v + 1], in1=dst,
                        op0=MULT, op1=ADD)

            xpw_all = pc.tile([128, NBLK * 96], BF, tag="xpw", name="xpw")
            load_select(xpw_all, xp_c, (0, 128))
            xpw = [xpw_all[:, D * 96:(D + 1) * 96] for D in range(NBLK)]
            dtw = pc.tile([DT_RANK, DL], BF, tag="dtw", name="dtw")
            load_select(dtw, dw_c, (0, DT_RANK))
            cst = pc.tile([128, 192], F32, tag="cst", name="cst")
            load_select(cst, cs_c, (0, 128))
            convw = cst[:, 0:32]
            convb = cst[:, 32:40]
            szb = cst[:, 40:48]
            dtb = cst[:, 48:56]
            acol = cst[:, 56:184]
            dcol = cst[:, 184:192]

            ones_m = pc.tile([128, 128], BF, tag="ones", name="ones")
            nc.vector.memset(ones_m, 1.0 / D_MODEL)
            epsb = pc.tile([128, 1], F32, tag="epsb", name="epsb")
            nc.vector.memset(epsb, 1e-5)

            # ---- x: AllGather quarters ----
            nc.sync.dma_start(out=xg_src[:], in_=xq[:])
            if sim:
                for p in range(4):
                    nc.sync.dma_start(out=xg[p * D_MODEL:(p + 1) * D_MODEL, :],
                                      in_=xg_src[:])
            else:
                nc.gpsimd.collective_compute(
                    "AllGather", mybir.AluOpType.bypass,
                    replica_groups=[[0, 1, 2, 3], [4, 5, 6, 7]],
                    ins=[xg_src[:]], outs=[xg[:]])

            # ---- phase 1: LN stats on raw x; xn = flip-select(normalized) ----
            xn = []
            with tc.tile_pool(name="p1", bufs=1) as p1:
                raw = []
                for D in range(NBLK):
                    t = p1.tile([128, SEQ], BF, tag=f"raw{D}", name="raw")
                    for p in range(4):
                        nc.sync.dma_start(
                            out=t[:, p * 512:(p + 1) * 512],
                            in_=xg[p * D_MODEL + D * 128:
                                   p * D_MODEL + (D + 1) * 128, :])
                    raw.append(t)
                mur = p1.tile([128, SEQ], F32, tag="mur", name="mur")
                rstd = p1.tile([128, SEQ], F32, tag="rstd", name="rstd")
                with tc.tile_pool(name="psA", bufs=1, space="PSUM") as psA:
                    mu_ps = psA.tile([128, SEQ], F32, tag="mu", name="mu")
                    ex2_ps = psA.tile([128, SEQ], F32, tag="ex2", name="ex2")
                    for D in range(NBLK):
                        xsq = p1.tile([128, SEQ], BF, tag="xsq", name="xsq",
                                      bufs=1)
                        nc.gpsimd.tensor_mul(xsq, raw[D], raw[D])
                        for c in range(NTC):
                            sl = bass.ts(c, 512)
                            nc.tensor.matmul(mu_ps[:, sl], ones_m[:],
                                             raw[D][:, sl],
                                             start=(D == 0), stop=(D == NBLK - 1))
                            nc.tensor.matmul(ex2_ps[:, sl], ones_m[:],
                                             xsq[:, sl],
                                             start=(D == 0), stop=(D == NBLK - 1))
                    nc.scalar.activation(mur, mu_ps[:], AF.Copy)
                    nc.vector.tensor_mul(rstd, mur, mur)
                    nc.vector.tensor_sub(rstd, ex2_ps[:], rstd)
                nc.scalar.activation(rstd, rstd, AF.Sqrt, bias=epsb[:, 0:1])
                nc.vector.reciprocal(rstd, rstd)

                for D in range(NBLK):
                    tn = p1.tile([128, SEQ], BF, tag="tn", name="tn", bufs=2)
                    nc.vector.tensor_sub(tn, raw[D], mur)
                    nc.vector.tensor_mul(tn, tn, rstd)
                    xs_t = pxn.tile([128, SEQ], BF, tag="xn", name="xn")
                    nc.vector.tensor_scalar(xs_t, _rev(tn[:, :]), s_b, None,
                                            op0=MULT)
                    nc.vector.scalar_tensor_tensor(
                        out=xs_t, in0=tn, scalar=s_f, in1=xs_t,
                        op0=MULT, op1=ADD)
                    xn.append(xs_t)

            # ---- phase 2: in_proj ----
            xr = []
            with tc.tile_pool(name="p2", bufs=1) as p2:
                winT = []
                for D in range(NBLK):
                    t = p2.tile([128, 2 * DL], BF, tag=f"wi{D}", name="wi")
                    load_select(t, wi_c, (D * 128, (D + 1) * 128))
                    winT.append(t)
                with tc.tile_pool(name="psB", bufs=4, space="PSUM") as psB:
                    for m in range(16):
                        if m < NBLK:
                            xt = pchain.tile([128, 3 + SEQ], BF, tag="chain",
                                             name="chain")
                            nc.vector.memset(xt[:, 0:3], 0.0)
                            xr.append(xt)
                        for cc in range(2):
                            pxz = psB.tile([128, 1024], F32, tag="ps", name="ps")
                            for half in range(2):
                                sl = bass.ts(2 * cc + half, 512)
                                for D in range(NBLK):
                                    nc.tensor.matmul(
                                        pxz[:, bass.ts(half, 512)],
                                        winT[D][:, bass.ts(m, 128)],
                                        xn[D][:, sl],
                                        start=(D == 0), stop=(D == NBLK - 1))
                            if m < NBLK:
                                nc.scalar.activation(
                                    xr[m][:, 3 + cc * 1024:3 + (cc + 1) * 1024],
                                    pxz[:], AF.Copy)
                            else:
                                gst = pmisc.tile([128, 1024], BF, tag="gst",
                                                 name="gst", bufs=2)
                                nc.scalar.activation(
                                    gst, pxz[:], AF.Silu,
                                    bias=szb[:, m - NBLK:m - NBLK + 1])
                                nc.sync.dma_start(
                                    out=gate_dram[(m - NBLK) * 128:
                                                  (m - NBLK + 1) * 128,
                                                  bass.ts(cc, 1024)],
                                    in_=gst)

            # ---- phase 3: causal depthwise conv + silu -> u ----
            u = []
            for D in range(NBLK):
                acc = pmisc.tile([128, SEQ], BF, tag="cacc", name="cacc", bufs=1)
                nc.vector.tensor_scalar(acc, xr[D][:, 0:SEQ],
                                        convw[:, 4 * D:4 * D + 1], None, op0=MULT)
                for k in range(1, D_CONV):
                    nc.vector.scalar_tensor_tensor(
                        out=acc, in0=xr[D][:, k:k + SEQ],
                        scalar=convw[:, 4 * D + k:4 * D + k + 1], in1=acc,
                        op0=MULT, op1=ADD)
                ut = pchain.tile([128, SEQ], BF, tag="chain", name="chain")
                nc.scalar.activation(ut, acc, AF.Silu, bias=convb[:, D:D + 1])
                u.append(ut)

            # ---- phases 4-9 ----
            with (
                tc.tile_pool(name="p5", bufs=1) as p5,
                tc.tile_pool(name="pxs", bufs=8) as pxs,      # dtu -> ysel
                tc.tile_pool(name="py", bufs=8) as py,        # yac
            ):
              with tc.tile_pool(name="psC", bufs=2, space="PSUM") as psC:
                # x_proj partial + pair AllReduce
                pdbc = psC.tile([128, SEQ], F32, tag="p4", name="p4")
                for c in range(NTC):
                    for D in range(NBLK):
                        nc.tensor.matmul(pdbc[0:96, bass.ts(c, 512)], xpw[D],
                                         u[D][:, bass.ts(c, 512)],
                                         start=(D == 0), stop=(D == NBLK - 1))
                dst = pmisc.tile([96, SEQ], BF, tag="dbcst", name="dbcst", bufs=1)
                nc.scalar.activation(dst, pdbc[0:96, :], AF.Copy)
                nc.sync.dma_start(out=cc_in[:], in_=dst)
                if sim:
                    nc.sync.dma_start(out=cc_out[:], in_=cc_in[:])
                else:
                    nc.gpsimd.collective_compute(
                        "AllReduce", ADD,
                        replica_groups=[[0, 1], [2, 3], [4, 5], [6, 7]],
                        ins=[cc_in[:]], outs=[cc_out[:]])
                # AR-independent work, overlaps the collective
                ebc = p5.tile([96, 32 * 128], BF, tag="ebc", name="ebc")
                nc.sync.dma_start(out=ebc, in_=ebc_c[:])
                yac = []
                for D in range(NBLK):
                    yt = py.tile([128, SEQ], BF, tag="y", name="y")
                    nc.vector.tensor_scalar(yt, u[D], dcol[:, D:D + 1], None,
                                            op0=MULT)
                    yac.append(yt)
                dbc = p5.tile([96, SEQ], BF, tag="dbc", name="dbc")
                nc.sync.dma_start(out=dbc, in_=cc_out[:])

                # dt = softplus(dtw@dbc + dtb) via exp series
                dt = []
                dtu = []
                for D in range(NBLK):
                    pdt = psC.tile([128, SEQ], F32, tag="p4", name="p4")
                    for c in range(NTC):
                        nc.tensor.matmul(pdt[:, bass.ts(c, 512)],
                                         dtw[:, bass.ts(D, 128)],
                                         dbc[0:DT_RANK, bass.ts(c, 512)],
                                         start=True, stop=True)
                    ex = pmisc.tile([128, SEQ], BF, tag="spx", name="spx", bufs=1)
                    nc.scalar.activation(ex, pdt[:], AF.Exp, bias=dtb[:, D:D + 1])
                    q = pmisc.tile([128, SEQ], BF, tag="q", name="q", bufs=1)
                    nc.vector.tensor_scalar(q, ex, -1.0 / 3.0, 0.5,
                                            op0=MULT, op1=ADD)
                    nc.vector.tensor_mul(q, ex, q)
                    nc.vector.tensor_scalar(q, q, -1.0, 1.0, op0=MULT, op1=ADD)
                    dtt = pchain.tile([128, SEQ], BF, tag="chain", name="chain")
                    nc.vector.tensor_mul(dtt, ex, q)
                    dt.append(dtt)
                    dut = pxs.tile([128, SEQ], BF, tag="xs", name="xs")
                    nc.vector.tensor_mul(dut, dtt, u[D])
                    dtu.append(dut)

                # selective scan
                with tc.tile_pool(name="pbc", bufs=2) as pbc:
                    for n in range(D_STATE):
                        pb = psC.tile([128, SEQ], F32, tag="p4", name="p4")
                        for c in range(NTC):
                            nc.tensor.matmul(pb[:, bass.ts(c, 512)],
                                             ebc[64:96, bass.ts(n, 128)],
                                             dbc[64:96, bass.ts(c, 512)],
                                             start=True, stop=True)
                        brep = pbc.tile([128, SEQ], BF, tag="brep", name="brep")
                        nc.scalar.activation(brep, pb[:], AF.Copy)
                        pcs = psC.tile([128, SEQ], F32, tag="p4", name="p4")
                        for c in range(NTC):
                            nc.tensor.matmul(pcs[:, bass.ts(c, 512)],
                                             ebc[64:96, bass.ts(16 + n, 128)],
                                             dbc[64:96, bass.ts(c, 512)],
                                             start=True, stop=True)
                        crep = pbc.tile([128, SEQ], BF, tag="crep", name="crep")
                        nc.scalar.activation(crep, pcs[:], AF.Copy)
                        for D in range(NBLK):
                            av = pxn.tile([128, SEQ], BF, tag="xn", name="xn")
                            nc.scalar.activation(
                                av, dt[D], AF.Exp,
                                scale=acol[:, D * D_STATE + n:
                                           D * D_STATE + n + 1])
                            bv = pxn.tile([128, SEQ], BF, tag="xn", name="xn")
                            nc.vector.tensor_mul(bv, dtu[D], brep)
                            nc.vector.tensor_tensor_scan(av, av, bv, 0.0,
                                                         op0=MULT, op1=ADD)
                            nc.vector.tensor_mul(bv, av, crep)
                            nc.gpsimd.tensor_add(yac[D], yac[D], bv)

              # ---- phase 8: gating + flip-select back to true time ----
              ysel = []
              for D in range(NBLK):
                    g = pgs.tile([128, SEQ], BF, tag="gs", name="gs")
                    nc.sync.dma_start(out=g,
                                      in_=gate_dram[D * 128:(D + 1) * 128, :])
                    nc.gpsimd.tensor_mul(yac[D], yac[D], g)
                    ys = pxs.tile([128, SEQ], BF, tag="xs", name="xs")
                    nc.vector.tensor_scalar(ys, _rev(yac[D][:, :]), s_b, None,
                                            op0=MULT)
                    nc.vector.scalar_tensor_tensor(
                        out=ys, in0=yac[D], scalar=s_f, in1=ys,
                        op0=MULT, op1=ADD)
                    ysel.append(ys)

              # ---- phase 9: fused out_proj @ proj ----
              with (
                    tc.tile_pool(name="p9", bufs=1) as p9,
                    tc.tile_pool(name="psD", bufs=4, space="PSUM") as psD,
              ):
                    wf = []
                    for D in range(NBLK):
                        t = p9.tile([128, D_MODEL], BF, tag=f"wf{D}", name="wf")
                        load_select(t, wf_c, (D * 128, (D + 1) * 128))
                        wf.append(t)
                    for m in range(NMT):
                        po = psD.tile([128, 1024], F32, tag="po", name="po")
                        for oc in range(2):
                            for D in range(NBLK):
                                nc.tensor.matmul(po[:, bass.ts(oc, 512)],
                                                 ysel[D][:, bass.ts(m, 128)],
                                                 wf[D][:, bass.ts(oc, 512)],
                                                 start=(D == 0),
                                                 stop=(D == NBLK - 1))
                        ot = pgs.tile([128, 1024], BF, tag="gs", name="gs")
                        nc.scalar.activation(ot, po[:], AF.Copy)
                        nc.sync.dma_start(
                            out=yrs_in[m * 128:(m + 1) * 128, :],
                            in_=ot)

            # ---- ReduceScatter y over each batch group ----
            if sim:
                nc.sync.dma_start(out=yrs_out[:], in_=yrs_in[0:512, :])
            else:
                nc.gpsimd.collective_compute(
                    "ReduceScatter", ADD,
                    replica_groups=[[0, 1, 2, 3], [4, 5, 6, 7]],
                    ins=[yrs_in[:]], outs=[yrs_out[:]])
            nc.sync.dma_start(out=y_part[:], in_=yrs_out[:])
    nc.compile()
    return nc


_CACHE = {}


def _weights_key(inputs):
    hs = hashlib.sha1()
    for k in sorted(inputs):
        if k != "x":
            hs.update(np.ascontiguousarray(np.asarray(inputs[k])).tobytes())
    return hs.hexdigest()


def _get_runner(inputs):
    key = _weights_key(inputs)
    if _CACHE.get("key") == key:
        return _CACHE["runner"]
    import jax
    from jax.sharding import Mesh, PartitionSpec
    from jax.experimental.shard_map import shard_map
    from concourse import bass2jax

    nc = _build_program(inputs)
    bass2jax.install_neuronx_cc_hook()

    partition_name = nc.partition_id_tensor.name if nc.partition_id_tensor else None
    in_names, out_names, out_avals, zero_outs = [], [], [], []
    for alloc in nc.m.functions[0].allocations:
        if not isinstance(alloc, mybir.MemoryLocationSet):
            continue
        name = alloc.memorylocations[0].name
        if alloc.kind == "ExternalInput":
            if name != partition_name:
                in_names.append(name)
        elif alloc.kind == "ExternalOutput":
            out_names.append(name)
            shape = tuple(alloc.tensor_shape)
            dtype = mybir.dt.np(alloc.dtype)
            out_avals.append(jax.core.ShapedArray(shape, dtype))
            zero_outs.append(np.zeros(shape, dtype))
    n_params = len(in_names)
    all_in_names = list(in_names) + list(out_names)
    if partition_name is not None:
        all_in_names.append(partition_name)

    def _body(*args):
        operands = list(args)
        if partition_name is not None:
            operands.append(bass2jax.partition_id_tensor())
        outs = bass2jax._bass_exec_p.bind(
            *operands,
            out_avals=tuple(out_avals),
            in_names=tuple(all_in_names),
            out_names=tuple(out_names),
            lowering_input_output_aliases=(),
            sim_require_finite=True,
            sim_require_nnan=True,
            nc=nc,
        )
        return tuple(outs)

    devices = jax.devices()[:8]
    mesh = Mesh(np.asarray(devices), ("core",))
    n_outs = len(out_avals)
    in_specs = (PartitionSpec("core"),) * (n_params + n_outs)
    out_specs = (PartitionSpec("core"),) * n_outs
    sharded = jax.jit(
        shard_map(_body, mesh=mesh, in_specs=in_specs, out_specs=out_specs,
                  check_rep=False),
        keep_unused=True)

    def prepare(maps):
        per_core = [[np.asarray(m[nm]) for nm in in_names] for m in maps]
        concat_in = [np.concatenate([per_core[c][i] for c in range(8)], axis=0)
                     for i in range(n_params)]
        concat_zeros = [np.zeros((8 * z.shape[0], *z.shape[1:]), z.dtype)
                        for z in zero_outs]
        return concat_in + concat_zeros

    def call(args):
        return sharded(*args)

    def to_results(out_arrs):
        return [
            {nm: np.asarray(out_arrs[i]).reshape(8, *out_avals[i].shape)[c]
             for i, nm in enumerate(out_names)}
            for c in range(8)
        ]

    def runner(maps):
        return to_results(call(prepare(maps)))

    runner.prepare = prepare
    runner.call = call
    runner.to_results = to_results
    _CACHE["key"] = key
    _CACHE["runner"] = runner
    return runner


def make_in_maps(inputs):
    x = np.asarray(inputs["x"], np.float32)
    maps = []
    for c in range(8):
        b, r, h = c // 4, (c // 2) % 2, c % 2
        p = c % 4
        xqv = np.ascontiguousarray(
            x[b, p * 512:(p + 1) * 512, :].T).astype(BF16)
        s = np.zeros((128, 8), np.float32)
        s[:, 2 * r + h] = 1.0
        s[:, 4] = 1.0 if r == 0 else 0.0
        s[:, 5] = 0.0 if r == 0 else 1.0
        maps.append({"xq": xqv, "sel": s})
    return maps


def gather(inputs, results):
    x = np.asarray(inputs["x"], np.float32)
    proj_b = np.asarray(inputs["proj_b"], np.float32)
    out = x + proj_b[None, None, :]
    for c in range(8):
        b, p = c // 4, c % 4
        part = np.asarray(results[c]["y_part"]).astype(np.float32)
        out[b, p * 512:(p + 1) * 512, :] += part
    return out


def kernel(**inputs) -> np.ndarray:
    runner = _get_runner(inputs)
    maps = make_in_maps(inputs)
    results = runner(maps)
    return gather(inputs, results)
